# revision 7
# baseline (speedup 1.0000x reference)
"""Trainium2 Bass kernel for ragged-sequence growing-prefix softmax attention.

Reference computation (T=131072 tokens, B=1024 ragged segments, D=512):
    s = context @ theta            # [T] scores; |s| <= ~0.07 for this data
    e = exp(s - segmax)            # segmax cancels exactly in the ratio
    out_t = segprefix(e*c)_t / segprefix(e)_t

Device strategy (8 cores, data parallel over segments):
  - 24 sub-slabs cut at segment boundaries near j*T/24 tokens; core c gets 3
    of them as independent carry chains (interleaved to hide carry latency).
  - Each sub-slab: 45 tiles of 127 tokens + carry row (row 0), 5 tiles per
    DMA group (10KB descriptors; small descriptors cap DMA queues ~50GB/s).
  - Host sends x as packed bf16 hi/lo pairs (same bytes as fp32) with a
    per-tile "ones" column. exp weights fold into the mask via per-partition
    tensor_scalar ops (fast 4x DVE mode, bf16 in/out):
        mhi[j,i] = (i>=j & i<=end_j) * e_hi_j     (e_hi = bf16(e))
        mlo[j,i] = (i>=j & i<=end_j) * e_lo_j     (e_lo = e - e_hi, fp32)
      num = mhi.T@x_hi + mhi.T@x_lo + mlo.T@x_hi  (mlo.x_lo ~2^-18, dropped)
      den = mhi.T@ones_hi + mhi.T@ones_lo + mlo.T@ones_hi
  - mask column 0 = (end_j==127)*e_j extracts the running sum of the segment
    open at the tile boundary into psum row 0 (no extra matmul); one ACT +
    one DVE op re-inject it (bf16 hi + exact lo compensation) as row 0 of the
    next tile's rhs; the carry-row mask weight is 1.0 (e32 row 0 forced).
  - scores: s = reduce(x_hi * theta) per group in bf16 (s error ~1e-4 ->
    output error well below the fp32 reference's own cancellation noise,
    which is max 5.2e-3 / p99 5.3e-4 vs float64).
"""
import numpy as np

T = 131072
B = 1024
D = 512
NCORES = 8
CHAINS = 3              # sub-slabs per core
NSUB = NCORES * CHAINS  # 24
TPT = 127               # tokens per tile (row 0 is the carry row)
SUBTILES = 45           # tiles per sub-slab
GT = 5                  # tiles per DMA group
NG = SUBTILES // GT     # 9 groups
CW = 520                # per-tile block: 512 x | 1 ones | 7 pad
W = GT * CW             # 2600 packed width per hi/lo half
NPAD = TPT * SUBTILES   # 5715 padded tokens per sub-slab

_CACHE = {}


def _patch_walrus_ldw_opt():
    """Enable walrus' redundant-LDWEIGHTS elimination so consecutive matmuls
    sharing one stationary operand skip the reload."""
    import concourse.bass_utils as bu
    if getattr(bu, "_ldw_patched", False):
        return
    orig = bu.run_command

    def patched(argv, **kw):
        pass  # ldw-opt patch disabled (walrus visitInstLdweights error)
        return orig(argv, **kw)

    bu.run_command = patched
    bu._ldw_patched = True


def _build_program():
    import concourse.bacc as bacc
    import concourse.tile as tile
    import concourse.mybir as mybir
    from contextlib import ExitStack

    _patch_walrus_ldw_opt()

    f32 = mybir.dt.float32
    bf16 = mybir.dt.bfloat16
    AF = mybir.ActivationFunctionType
    ALU = mybir.AluOpType

    nc = bacc.Bacc("TRN2", target_bir_lowering=False, debug=False)

    x_d = [nc.dram_tensor(f"x{ch}", [NG, 128, 2 * W], bf16, kind="ExternalInput")
           for ch in range(CHAINS)]
    e_d = [nc.dram_tensor(f"end{ch}", [128, SUBTILES], f32, kind="ExternalInput")
           for ch in range(CHAINS)]
    iota_d = nc.dram_tensor("iota_mod", [128, 128], bf16, kind="ExternalInput")
    th_d = nc.dram_tensor("thetab", [128, W], bf16, kind="ExternalInput")
    y_d = [nc.dram_tensor(f"y{ch}", [NG, 128, GT * D], f32, kind="ExternalOutput")
           for ch in range(CHAINS)]

    with tile.TileContext(nc) as tc, ExitStack() as ctx:
        cpool = ctx.enter_context(tc.tile_pool(name="consts", bufs=1))
        xpool = ctx.enter_context(tc.tile_pool(name="x", bufs=3))
        spool = ctx.enter_context(tc.tile_pool(name="scr", bufs=2))
        gpool = ctx.enter_context(tc.tile_pool(name="gsmall", bufs=4))
        mpool = ctx.enter_context(tc.tile_pool(name="mask", bufs=4))
        opool = ctx.enter_context(tc.tile_pool(name="out", bufs=3))
        pspool = ctx.enter_context(tc.tile_pool(name="ps", bufs=4, space="PSUM"))

        iota = cpool.tile([128, 128], bf16)
        nc.sync.dma_start(iota[:], iota_d.ap()[:])
        thetab = cpool.tile([128, W], bf16)
        nc.sync.dma_start(thetab[:], th_d.ap()[:])
        end_sb = [cpool.tile([128, SUBTILES], f32, name=f"end_sb{ch}",
                             tag=f"end{ch}") for ch in range(CHAINS)]
        for ch in range(CHAINS):
            nc.sync.dma_start(end_sb[ch][:], e_d[ch].ap()[:])

        prev = [None] * CHAINS   # previous tile's psum (carry source)

        for g in range(NG):
            for ch in range(CHAINS):
                xt = xpool.tile([128, 2 * W], bf16)
                nc.sync.dma_start(xt[:], x_d[ch].ap()[g])

                # scores for the group: s = sum(x_hi * theta) per tile block
                scr = spool.tile([128, W], bf16)
                nc.vector.tensor_tensor(scr[:], xt[:, 0:W], thetab[:],
                                        op=ALU.mult)
                s_g = gpool.tile([128, GT], f32, tag="sg")
                nc.vector.tensor_reduce(
                    s_g[:], scr[:].rearrange("p (t c) -> p t c", c=CW),
                    axis=mybir.AxisListType.X, op=ALU.add)
                e32 = gpool.tile([128, GT], f32, tag="e32")
                nc.scalar.activation(e32[:], s_g[:], AF.Exp)
                # carry pseudo-row weight is exactly 1.0
                nc.vector.memset(e32[0:1, :], 1.0)
                e_hi_b = gpool.tile([128, GT], bf16, tag="ehib")
                nc.scalar.copy(e_hi_b[:], e32[:])
                e_hi = gpool.tile([128, GT], f32, tag="ehi")
                nc.gpsimd.tensor_copy(e_hi[:], e_hi_b[:])
                e_lo = gpool.tile([128, GT], f32, tag="elo")
                nc.vector.tensor_tensor(e_lo[:], e32[:], e_hi_b[:],
                                        op=ALU.subtract)

                y_g = opool.tile([128, GT * D], f32)

                for t in range(GT):
                    k = GT * g + t
                    xhi = xt[:, t * CW: t * CW + D]
                    ones_hi = xt[:, t * CW + D: t * CW + D + 1]
                    xlo = xt[:, W + t * CW: W + t * CW + D]
                    ones_lo = xt[:, W + t * CW + D: W + t * CW + D + 1]
                    ehc = e_hi[:, t: t + 1]
                    elc = e_lo[:, t: t + 1]
                    endc = end_sb[ch][:, k: k + 1]

                    # carry inject from previous tile of this chain
                    if prev[ch] is not None:
                        ppsum = prev[ch]
                        nc.scalar.copy(xt[0:1, t * CW: t * CW + D + 1],
                                       ppsum[0:1, 0: D + 1])
                        nc.vector.tensor_tensor(
                            xt[0:1, W + t * CW: W + t * CW + D + 1],
                            ppsum[0:1, 0: D + 1],
                            xt[0:1, t * CW: t * CW + D + 1],
                            op=ALU.subtract)

                    # masks with e folded in; column 0 = carry extraction
                    mhi = mpool.tile([128, 128], bf16, tag="mhi")
                    nc.gpsimd.tensor_scalar(mhi[:], iota[:], endc, ehc,
                                            op0=ALU.is_le, op1=ALU.mult)
                    mlo = mpool.tile([128, 128], bf16, tag="mlo")
                    nc.vector.tensor_scalar(mlo[:], iota[:], endc, elc,
                                            op0=ALU.is_le, op1=ALU.mult)
                    nc.gpsimd.tensor_scalar(mhi[:, 0:1], endc, 127.0, ehc,
                                            op0=ALU.is_equal, op1=ALU.mult)
                    nc.gpsimd.tensor_scalar(mlo[:, 0:1], endc, 127.0, elc,
                                            op0=ALU.is_equal, op1=ALU.mult)

                    # psum: [:, 0:512] num, [:, 512:513] den (adjacent banks,
                    # so the carry inject reads [0:513] in one AP)
                    psum = pspool.tile([128, 1024], f32)
                    nc.tensor.matmul(psum[:, 0:D], lhsT=mhi[:], rhs=xhi,
                                     start=True, stop=False)
                    nc.tensor.matmul(psum[:, 0:D], lhsT=mhi[:], rhs=xlo,
                                     start=False, stop=False)
                    nc.tensor.matmul(psum[:, D:D + 1], lhsT=mhi[:], rhs=ones_hi,
                                     start=True, stop=False)
                    nc.tensor.matmul(psum[:, D:D + 1], lhsT=mhi[:], rhs=ones_lo,
                                     start=False, stop=False)
                    nc.tensor.matmul(psum[:, 0:D], lhsT=mlo[:], rhs=xhi,
                                     start=False, stop=True)
                    nc.tensor.matmul(psum[:, D:D + 1], lhsT=mlo[:], rhs=ones_hi,
                                     start=False, stop=True)
                    prev[ch] = psum

                    rec = gpool.tile([128, 1], f32, tag="rec")
                    nc.vector.reciprocal(rec[:], psum[:, D:D + 1])
                    nc.scalar.activation(y_g[:, t * D:(t + 1) * D],
                                         psum[:, 0:D], AF.Copy, scale=rec[:])

                nc.scalar.dma_start(y_d[ch].ap()[g], y_g[:])

    nc.compile()
    return nc


def _bounds(lengths):
    cum = np.cumsum(lengths)
    assert cum[-1] == T
    bounds = [0]
    for j in range(1, NSUB):
        tgt = j * (T // NSUB)
        i = np.searchsorted(cum, tgt)
        lo = cum[i - 1] if i > 0 else 0
        hi = cum[i]
        bounds.append(int(lo if tgt - lo <= hi - tgt else hi))
    bounds.append(T)
    return bounds, cum


def _shard(context, lengths, theta):
    """Per-core input maps: packed bf16 hi/lo x groups, end tables, consts."""
    import ml_dtypes

    bounds, cum = _bounds(lengths)
    seg_end = np.repeat(cum - 1, lengths)     # [T] global last token of own seg

    jj = np.arange(128)
    iota_mod = np.where(jj[None, :] >= jj[:, None],
                        jj[None, :], 512).astype(ml_dtypes.bfloat16)

    thetab = np.zeros((128, W), dtype=ml_dtypes.bfloat16)
    th = theta.reshape(-1).astype(ml_dtypes.bfloat16)
    for t in range(GT):
        thetab[:, t * CW: t * CW + D] = th[None, :]

    in_maps = []
    slabs = []
    for c in range(NCORES):
        im = {"thetab": thetab, "iota_mod": iota_mod}
        for ch in range(CHAINS):
            u = CHAINS * c + ch
            b0, b1 = bounds[u], bounds[u + 1]
            n = b1 - b0
            assert n <= NPAD, (u, n)
            slabs.append((b0, n))

            x_ext = np.zeros((1 + NPAD, D), dtype=np.float32)
            x_ext[1:1 + n] = context[b0:b1]
            # tile k row p holds token 127k + p - 1 -> x_ext row 127k + p
            rows = (TPT * np.arange(SUBTILES))[:, None] + jj[None, :]
            xg = x_ext[rows]                          # [45, 128, 512] fp32
            x_hi = xg.astype(ml_dtypes.bfloat16)
            x_lo = (xg - x_hi.astype(np.float32)).astype(ml_dtypes.bfloat16)

            xpk = np.zeros((NG, 128, 2 * W), dtype=ml_dtypes.bfloat16)
            hi = xpk[:, :, 0:W].reshape(NG, 128, GT, CW)
            lo = xpk[:, :, W:2 * W].reshape(NG, 128, GT, CW)
            hi[:, :, :, 0:D] = x_hi.reshape(NG, GT, 128, D).transpose(0, 2, 1, 3)
            lo[:, :, :, 0:D] = x_lo.reshape(NG, GT, 128, D).transpose(0, 2, 1, 3)
            hi[:, :, :, D] = 1.0

            loc_end = np.empty(NPAD + 1, dtype=np.int64)
            loc_end[0] = -1
            loc_end[1:1 + n] = seg_end[b0:b1] - b0
            loc_end[1 + n:] = np.arange(n, NPAD)
            k_arr = np.arange(SUBTILES)
            idx = TPT * k_arr[None, :] + jj[:, None]
            end_all = np.minimum(loc_end[idx] + 1 - TPT * k_arr[None, :],
                                 127).astype(np.float32)

            im[f"x{ch}"] = xpk
            im[f"end{ch}"] = end_all
        in_maps.append(im)
    return in_maps, slabs


def kernel(context, context_theta, lengths, seg_ids):
    from concourse.bass_utils import run_bass_kernel_spmd

    context = np.asarray(context, dtype=np.float32)
    theta = np.asarray(context_theta, dtype=np.float32)
    lengths = np.asarray(lengths).astype(np.int64)

    if "nc" not in _CACHE:
        _CACHE["nc"] = _build_program()
    nc = _CACHE["nc"]

    in_maps, slabs = _shard(context, lengths, theta)
    res = run_bass_kernel_spmd(nc, in_maps, list(range(NCORES)))
    _CACHE["last_results"] = res

    out = np.empty((T, D), dtype=np.float32)
    for c in range(NCORES):
        for ch in range(CHAINS):
            b0, n = slabs[CHAINS * c + ch]
            ypk = res.results[c][f"y{ch}"]            # [NG, 128, GT*D]
            y = ypk.reshape(NG, 128, GT, D).transpose(0, 2, 1, 3)
            y = y.reshape(SUBTILES, 128, D)[:, 1:, :].reshape(NPAD, D)
            out[b0:b0 + n] = y[:n]
    return out


# revision 8
# speedup vs baseline: 1.1207x; 1.1207x over previous
"""Trainium2 Bass kernel for ragged-sequence growing-prefix softmax attention.

Reference computation (T=131072 tokens, B=1024 ragged segments, D=512):
    s = context @ theta            # [T] scores; |s| <= ~0.07 for this data
    e = exp(s - segmax)            # segmax cancels exactly in the ratio
    out_t = segprefix(e*c)_t / segprefix(e)_t

Device strategy (8 cores, data parallel over segments):
  - 24 sub-slabs cut at segment boundaries near j*T/24 tokens; core c gets 3
    of them as independent carry chains (interleaved to hide carry latency).
  - Each sub-slab: 45 tiles of 127 tokens + carry row (row 0), 5 tiles per
    DMA group (10KB descriptors; small descriptors cap DMA queues ~50GB/s).
  - Host sends x as packed bf16 hi/lo pairs (same bytes as fp32) with a
    per-tile "ones" column. exp weights fold into the mask via per-partition
    tensor_scalar ops (fast 4x DVE mode, bf16 in/out):
        mb[j,i] = bf16( (i>=j & i<=end_j) * e_j )
      num = mb.T@x_hi + mb.T@x_lo ; den = mb.T@ones
      (num and den share the SAME bf16-rounded weights, so the weight
      rounding largely cancels in num/den; residual ~1e-4-class, below the
      reference's own p99 cancellation noise)
  - mask column 0 = (end_j==127)*e_j extracts the running sum of the segment
    open at the tile boundary into psum row 0 (no extra matmul); one ACT +
    one DVE op re-inject it (bf16 hi + exact lo compensation) as row 0 of the
    next tile's rhs; the carry-row mask weight is 1.0 (e32 row 0 forced).
  - scores: s = reduce(x_hi * theta) per group in bf16 (s error ~1e-4 ->
    output error well below the fp32 reference's own cancellation noise,
    which is max 5.2e-3 / p99 5.3e-4 vs float64).
"""
import numpy as np

T = 131072
B = 1024
D = 512
NCORES = 8
CHAINS = 3              # sub-slabs per core
NSUB = NCORES * CHAINS  # 24
TPT = 127               # tokens per tile (row 0 is the carry row)
SUBTILES = 45           # tiles per sub-slab
GT = 5                  # tiles per DMA group
NG = SUBTILES // GT     # 9 groups
CW = 520                # per-tile block: 512 x | 1 ones | 7 pad
W = GT * CW             # 2600 packed width per hi/lo half
NPAD = TPT * SUBTILES   # 5715 padded tokens per sub-slab

_CACHE = {}


def _patch_walrus_ldw_opt():
    """Enable walrus' redundant-LDWEIGHTS elimination so consecutive matmuls
    sharing one stationary operand skip the reload."""
    import concourse.bass_utils as bu
    if getattr(bu, "_ldw_patched", False):
        return
    orig = bu.run_command

    def patched(argv, **kw):
        pass  # ldw-opt patch disabled (walrus visitInstLdweights error)
        return orig(argv, **kw)

    bu.run_command = patched
    bu._ldw_patched = True


def _build_program():
    import concourse.bacc as bacc
    import concourse.tile as tile
    import concourse.mybir as mybir
    from contextlib import ExitStack

    _patch_walrus_ldw_opt()

    f32 = mybir.dt.float32
    bf16 = mybir.dt.bfloat16
    AF = mybir.ActivationFunctionType
    ALU = mybir.AluOpType

    nc = bacc.Bacc("TRN2", target_bir_lowering=False, debug=False)

    x_d = [nc.dram_tensor(f"x{ch}", [NG, 128, 2 * W], bf16, kind="ExternalInput")
           for ch in range(CHAINS)]
    e_d = [nc.dram_tensor(f"end{ch}", [128, SUBTILES], f32, kind="ExternalInput")
           for ch in range(CHAINS)]
    iota_d = nc.dram_tensor("iota_mod", [128, 128], f32, kind="ExternalInput")
    th_d = nc.dram_tensor("thetab", [128, W], bf16, kind="ExternalInput")
    y_d = [nc.dram_tensor(f"y{ch}", [NG, 128, GT * D], f32, kind="ExternalOutput")
           for ch in range(CHAINS)]

    with tile.TileContext(nc) as tc, ExitStack() as ctx:
        cpool = ctx.enter_context(tc.tile_pool(name="consts", bufs=1))
        xpool = ctx.enter_context(tc.tile_pool(name="x", bufs=3))
        spool = ctx.enter_context(tc.tile_pool(name="scr", bufs=2))
        gpool = ctx.enter_context(tc.tile_pool(name="gsmall", bufs=4))
        mpool = ctx.enter_context(tc.tile_pool(name="mask", bufs=4))
        opool = ctx.enter_context(tc.tile_pool(name="out", bufs=3))
        pspool = ctx.enter_context(tc.tile_pool(name="ps", bufs=4, space="PSUM"))

        iota = cpool.tile([128, 128], f32)
        nc.sync.dma_start(iota[:], iota_d.ap()[:])
        thetab = cpool.tile([128, W], bf16)
        nc.sync.dma_start(thetab[:], th_d.ap()[:])
        end_sb = [cpool.tile([128, SUBTILES], f32, name=f"end_sb{ch}",
                             tag=f"end{ch}") for ch in range(CHAINS)]
        for ch in range(CHAINS):
            nc.sync.dma_start(end_sb[ch][:], e_d[ch].ap()[:])

        prev = [None] * CHAINS   # previous tile's psum (carry source)

        for g in range(NG):
            for ch in range(CHAINS):
                xt = xpool.tile([128, 2 * W], bf16)
                nc.sync.dma_start(xt[:], x_d[ch].ap()[g])

                # scores for the group: s = sum(x_hi * theta) per tile block
                scr = spool.tile([128, W], bf16)
                nc.vector.tensor_tensor(scr[:], xt[:, 0:W], thetab[:],
                                        op=ALU.mult)
                s_g = gpool.tile([128, GT], f32, tag="sg")
                nc.vector.tensor_reduce(
                    s_g[:], scr[:].rearrange("p (t c) -> p t c", c=CW),
                    axis=mybir.AxisListType.X, op=ALU.add)
                e32 = gpool.tile([128, GT], f32, tag="e32")
                nc.scalar.activation(e32[:], s_g[:], AF.Exp)
                # carry pseudo-row weight is exactly 1.0
                nc.vector.memset(e32[0:1, :], 1.0)

                y_g = opool.tile([128, GT * D], f32)

                for t in range(GT):
                    k = GT * g + t
                    xhi = xt[:, t * CW: t * CW + D]
                    ones_hi = xt[:, t * CW + D: t * CW + D + 1]
                    xlo = xt[:, W + t * CW: W + t * CW + D]
                    ones_lo = xt[:, W + t * CW + D: W + t * CW + D + 1]
                    ecol = e32[:, t: t + 1]
                    endc = end_sb[ch][:, k: k + 1]

                    # carry inject from previous tile of this chain
                    if prev[ch] is not None:
                        ppsum = prev[ch]
                        nc.scalar.copy(xt[0:1, t * CW: t * CW + D + 1],
                                       ppsum[0:1, 0: D + 1])
                        nc.vector.tensor_tensor(
                            xt[0:1, W + t * CW: W + t * CW + D + 1],
                            ppsum[0:1, 0: D + 1],
                            xt[0:1, t * CW: t * CW + D + 1],
                            op=ALU.subtract)

                    # e-folded mask (fp32), column 0 = carry extraction,
                    # then one bf16 cast for the matmul lhsT
                    maske = mpool.tile([128, 128], f32, tag="maske")
                    if t % 2 == 0:
                        nc.vector.tensor_scalar(maske[:], iota[:], endc, ecol,
                                                op0=ALU.is_le, op1=ALU.mult)
                    else:
                        nc.gpsimd.tensor_scalar(maske[:], iota[:], endc, ecol,
                                                op0=ALU.is_le, op1=ALU.mult)
                    nc.gpsimd.tensor_scalar(maske[:, 0:1], endc, 127.0, ecol,
                                            op0=ALU.is_equal, op1=ALU.mult)
                    mb = mpool.tile([128, 128], bf16, tag="mb")
                    nc.gpsimd.tensor_copy(mb[:], maske[:])

                    # psum: [:, 0:512] num, [:, 512:513] den (adjacent banks,
                    # so the carry inject reads [0:513] in one AP)
                    psum = pspool.tile([128, 1024], f32)
                    nc.tensor.matmul(psum[:, 0:D], lhsT=mb[:], rhs=xhi,
                                     start=True, stop=False)
                    nc.tensor.matmul(psum[:, 0:D], lhsT=mb[:], rhs=xlo,
                                     start=False, stop=True)
                    nc.tensor.matmul(psum[:, D:D + 1], lhsT=mb[:], rhs=ones_hi,
                                     start=True, stop=False)
                    nc.tensor.matmul(psum[:, D:D + 1], lhsT=mb[:], rhs=ones_lo,
                                     start=False, stop=True)
                    prev[ch] = psum

                    rec = gpool.tile([128, 1], f32, tag="rec")
                    nc.vector.reciprocal(rec[:], psum[:, D:D + 1])
                    nc.scalar.activation(y_g[:, t * D:(t + 1) * D],
                                         psum[:, 0:D], AF.Copy, scale=rec[:])

                nc.scalar.dma_start(y_d[ch].ap()[g], y_g[:])

    nc.compile()
    return nc


def _bounds(lengths):
    cum = np.cumsum(lengths)
    assert cum[-1] == T
    bounds = [0]
    for j in range(1, NSUB):
        tgt = j * (T // NSUB)
        i = np.searchsorted(cum, tgt)
        lo = cum[i - 1] if i > 0 else 0
        hi = cum[i]
        bounds.append(int(lo if tgt - lo <= hi - tgt else hi))
    bounds.append(T)
    return bounds, cum


def _shard(context, lengths, theta):
    """Per-core input maps: packed bf16 hi/lo x groups, end tables, consts."""
    import ml_dtypes

    bounds, cum = _bounds(lengths)
    seg_end = np.repeat(cum - 1, lengths)     # [T] global last token of own seg

    jj = np.arange(128)
    iota_mod = np.where(jj[None, :] >= jj[:, None],
                        jj[None, :], 512).astype(np.float32)

    thetab = np.zeros((128, W), dtype=ml_dtypes.bfloat16)
    th = theta.reshape(-1).astype(ml_dtypes.bfloat16)
    for t in range(GT):
        thetab[:, t * CW: t * CW + D] = th[None, :]

    in_maps = []
    slabs = []
    for c in range(NCORES):
        im = {"thetab": thetab, "iota_mod": iota_mod}
        for ch in range(CHAINS):
            u = CHAINS * c + ch
            b0, b1 = bounds[u], bounds[u + 1]
            n = b1 - b0
            assert n <= NPAD, (u, n)
            slabs.append((b0, n))

            x_ext = np.zeros((1 + NPAD, D), dtype=np.float32)
            x_ext[1:1 + n] = context[b0:b1]
            # tile k row p holds token 127k + p - 1 -> x_ext row 127k + p
            rows = (TPT * np.arange(SUBTILES))[:, None] + jj[None, :]
            xg = x_ext[rows]                          # [45, 128, 512] fp32
            x_hi = xg.astype(ml_dtypes.bfloat16)
            x_lo = (xg - x_hi.astype(np.float32)).astype(ml_dtypes.bfloat16)

            xpk = np.zeros((NG, 128, 2 * W), dtype=ml_dtypes.bfloat16)
            hi = xpk[:, :, 0:W].reshape(NG, 128, GT, CW)
            lo = xpk[:, :, W:2 * W].reshape(NG, 128, GT, CW)
            hi[:, :, :, 0:D] = x_hi.reshape(NG, GT, 128, D).transpose(0, 2, 1, 3)
            lo[:, :, :, 0:D] = x_lo.reshape(NG, GT, 128, D).transpose(0, 2, 1, 3)
            hi[:, :, :, D] = 1.0

            loc_end = np.empty(NPAD + 1, dtype=np.int64)
            loc_end[0] = -1
            loc_end[1:1 + n] = seg_end[b0:b1] - b0
            loc_end[1 + n:] = np.arange(n, NPAD)
            k_arr = np.arange(SUBTILES)
            idx = TPT * k_arr[None, :] + jj[:, None]
            end_all = np.minimum(loc_end[idx] + 1 - TPT * k_arr[None, :],
                                 127).astype(np.float32)

            im[f"x{ch}"] = xpk
            im[f"end{ch}"] = end_all
        in_maps.append(im)
    return in_maps, slabs


def kernel(context, context_theta, lengths, seg_ids):
    from concourse.bass_utils import run_bass_kernel_spmd

    context = np.asarray(context, dtype=np.float32)
    theta = np.asarray(context_theta, dtype=np.float32)
    lengths = np.asarray(lengths).astype(np.int64)

    if "nc" not in _CACHE:
        _CACHE["nc"] = _build_program()
    nc = _CACHE["nc"]

    in_maps, slabs = _shard(context, lengths, theta)
    res = run_bass_kernel_spmd(nc, in_maps, list(range(NCORES)))
    _CACHE["last_results"] = res

    out = np.empty((T, D), dtype=np.float32)
    for c in range(NCORES):
        for ch in range(CHAINS):
            b0, n = slabs[CHAINS * c + ch]
            ypk = res.results[c][f"y{ch}"]            # [NG, 128, GT*D]
            y = ypk.reshape(NG, 128, GT, D).transpose(0, 2, 1, 3)
            y = y.reshape(SUBTILES, 128, D)[:, 1:, :].reshape(NPAD, D)
            out[b0:b0 + n] = y[:n]
    return out
